# revision 2
# baseline (speedup 1.0000x reference)
"""Causal self-attention TRN2 Bass kernel, v2 (instruction-count reduced).

Problem: B=4, T=2048, C=1024, H=16 heads, D=64 (fp32).

Sharding (8 cores): core i handles batch b = i//2 and heads
8*(i%2) .. 8*(i%2)+8  (8 heads, 512 features). Each core:
  qkv_local = x[b] @ W_attn[:, cols] (+bias)       [2048, 512] x3
  attention over its 8 heads (causal, T=2048)
  partial_out = y_local @ W_proj[rows, :]          [2048, 1024]
Host: out[b] = partial(core 2b) + partial(core 2b+1) + bias_terms.

v2 vs v1 (the device is dispatch-bound ~10-20us/instruction, so count
is everything):
  - q/k QKV matmuls use free dim 512 (whole chunk), not 256 halves.
  - q/k psum->sbuf moves ride on ACT (Identity + per-partition bias).
  - S head-pair packed in one [128,1024] 2-bank psum tile; ONE exp per
    (j,kb) with a strided AP covering both heads.
  - causal mask added by the PE itself: S += triT.T @ [I|I] fused into
    the S accumulation group (no DVE in the S->exp chain).
  - v psum->sbuf is one strided copy per 128-token block.
"""
import numpy as np
from contextlib import ExitStack

import jax
import concourse.bass as bass
import concourse.tile as tile
from concourse import bacc, mybir
from concourse.bass_utils import run_bass_kernel_spmd

jax.config.update("jax_compilation_cache_dir", "/tmp/jaxcache")
jax.config.update("jax_persistent_cache_min_entry_size_bytes", -1)
jax.config.update("jax_persistent_cache_min_compile_time_secs", 0.0)

B, T, C, H, D = 4, 2048, 1024, 16, 64
NCORES = 8
HPC = 8            # heads per core
FL = HPC * D       # 512 local features per core
NTC = 4            # 512-token chunks per core
NTB = 16           # 128-token blocks per core
F32 = mybir.dt.float32
F32R = mybir.dt.float32r
AF = mybir.ActivationFunctionType

_CACHED_NC = None


def _build(reps=1, weave=True, pace=True, attn=True, qkv=True, proj=True):
    # attn/qkv/proj=False build timing-isolation variants (wrong numerics).
    nc = bacc.Bacc("TRN2", target_bir_lowering=False, debug=False,
                   num_devices=NCORES)

    xt = nc.dram_tensor("xt", [C, T], F32, kind="ExternalInput").ap()
    wq = nc.dram_tensor("wq", [C, FL], F32, kind="ExternalInput").ap()
    wk = nc.dram_tensor("wk", [C, FL], F32, kind="ExternalInput").ap()
    wv = nc.dram_tensor("wv", [C, FL], F32, kind="ExternalInput").ap()
    wp = nc.dram_tensor("wp", [FL, C], F32, kind="ExternalInput").ap()
    bq = nc.dram_tensor("bq", [128, 4], F32, kind="ExternalInput").ap()
    bk = nc.dram_tensor("bk", [128, 4], F32, kind="ExternalInput").ap()
    triT = nc.dram_tensor("triT", [128, 128], F32, kind="ExternalInput").ap()
    id2 = nc.dram_tensor("id2", [128, 256], F32, kind="ExternalInput").ap()
    out = nc.dram_tensor("out", [T, C], F32, kind="ExternalOutput").ap()

    with tile.TileContext(nc) as tc, ExitStack() as ctx:
        ctx.enter_context(nc.allow_low_precision(reason="fp32r matmuls"))
        singles = ctx.enter_context(tc.tile_pool(name="singles", bufs=1))
        xt_pool = ctx.enter_context(tc.tile_pool(name="xt", bufs=10))
        qT_pool = ctx.enter_context(tc.tile_pool(name="qT", bufs=8))
        attT_pool = ctx.enter_context(tc.tile_pool(name="attT", bufs=3))
        yT_pool = ctx.enter_context(tc.tile_pool(name="yT", bufs=2))
        rc_pool = ctx.enter_context(tc.tile_pool(name="rc", bufs=2))
        bcs_pool = ctx.enter_context(tc.tile_pool(name="bcs", bufs=2))
        o_pool = ctx.enter_context(tc.tile_pool(name="o", bufs=2))
        ps_a = ctx.enter_context(tc.tile_pool(name="ps_a", bufs=2, space="PSUM"))
        ps_s = ctx.enter_context(tc.tile_pool(name="ps_s", bufs=2, space="PSUM"))
        ps_y = ctx.enter_context(tc.tile_pool(name="ps_y", bufs=2, space="PSUM"))

        wq_sb = singles.tile([128, 8, FL], F32R)
        wk_sb = singles.tile([128, 8, FL], F32R)
        wv_sb = singles.tile([128, 8, FL], F32R)
        wp_sb = singles.tile([128, 4, C], F32R)
        wq_r = wq.rearrange("(cc p) f -> p cc f", p=128).bitcast(F32R)
        wk_r = wk.rearrange("(cc p) f -> p cc f", p=128).bitcast(F32R)
        wv_r = wv.rearrange("(cc p) f -> p cc f", p=128).bitcast(F32R)
        wp_r = wp.rearrange("(j p) o -> p j o", p=128).bitcast(F32R)
        for cc in range(8):
            nc.scalar.dma_start(out=wq_sb[:, cc], in_=wq_r[:, cc])
            nc.scalar.dma_start(out=wk_sb[:, cc], in_=wk_r[:, cc])
            nc.scalar.dma_start(out=wv_sb[:, cc], in_=wv_r[:, cc])
        for j in range(4):
            nc.scalar.dma_start(out=wp_sb[:, j], in_=wp_r[:, j])
        bq_sb = singles.tile([128, 4], F32)
        bk_sb = singles.tile([128, 4], F32)
        triT_sb = singles.tile([128, 128], F32R)
        id2_sb = singles.tile([128, 256], F32R)
        nc.scalar.dma_start(out=bq_sb, in_=bq)
        nc.scalar.dma_start(out=bk_sb, in_=bk)
        nc.scalar.dma_start(out=triT_sb, in_=triT.bitcast(F32R))
        nc.scalar.dma_start(out=id2_sb, in_=id2.bitcast(F32R))

        # kT: [128 (pair-feature), j (head pair), t]
        kT_sb = singles.tile([128, 4, T], F32R)
        # v: [128 (t%128), tb, head, 66]; cols 64,65 stay 1.0 -> denominator
        v_sb = singles.tile([128, NTB, HPC, 66], F32)
        nc.vector.memset(v_sb, 1.0)

        def _run_proj(tcx, yt, rep):
            if not proj:
                return
            for tb_rel in range(4):
                for oc in range(2):
                    pp = ps_a.tile([128, 512], F32, tag="a", name="pp")
                    for j in range(4):
                        nc.tensor.matmul(
                            pp, yt[:, j, tb_rel * 128:(tb_rel + 1) * 128],
                            wp_sb[:, j, oc * 512:(oc + 1) * 512],
                            start=(j == 0), stop=(j == 3))
                    po = o_pool.tile([128, 512], F32, tag="o",
                                     name=f"po{rep}_{tcx}_{tb_rel}_{oc}")
                    nc.vector.tensor_copy(po, pp)
                    nc.sync.dma_start(
                        out=out[tcx * 512 + tb_rel * 128:
                                tcx * 512 + (tb_rel + 1) * 128,
                                oc * 512:(oc + 1) * 512],
                        in_=po)

        for rep in range(reps):
            if rep > 0:
                tc.strict_bb_all_engine_barrier()

            qts_all = {}

            def a_units(tcx, rep=rep, qts_all=qts_all):
                """Yield thunks; each emits one QKV work unit for chunk tcx."""
                qts = qts_all[tcx] = [
                    qT_pool.tile([128, 512], F32R, tag="qT",
                                 name=f"qt{rep}_{tcx}_{j}")
                    for j in range(4)
                ]
                t0 = tcx * 512
                xts = [xt_pool.tile([128, 512], F32R, tag="xt",
                                    name=f"xt{rep}_{tcx}_{cc}")
                       for cc in range(8)]

                def dmas(xts=xts, t0=t0):
                    for cc in range(8):
                        nc.sync.dma_start(
                            out=xts[cc],
                            in_=xt[cc * 128:(cc + 1) * 128,
                                   t0:t0 + 512].bitcast(F32R))
                yield dmas

                def qk_group(w_sb, dest, bias, j):
                    p = ps_a.tile([128, 512], F32, tag="a", name="pqk")
                    for cc in range(8):
                        nc.tensor.matmul(
                            p, w_sb[:, cc, j * 128:(j + 1) * 128],
                            xts[cc], start=(cc == 0), stop=(cc == 7))
                    nc.scalar.activation(dest, p, AF.Identity, bias=bias)

                for j in range(4):
                    def uq(j=j, qts=qts, qk=qk_group):
                        qk(wq_sb, qts[j], bq_sb[:, j:j + 1], j)
                    yield uq

                    def uk(j=j, t0=t0, qk=qk_group):
                        qk(wk_sb, kT_sb[:, j, t0:t0 + 512],
                           bk_sb[:, j:j + 1], j)
                    yield uk

                def v_group(tb_rel, xts=xts, tcx=tcx):
                    tb = tcx * 4 + tb_rel
                    pv = ps_a.tile([128, 512], F32, tag="a", name="pv")
                    for cc in range(8):
                        nc.tensor.matmul(
                            pv, xts[cc][:, tb_rel * 128:(tb_rel + 1) * 128],
                            wv_sb[:, cc, :], start=(cc == 0), stop=(cc == 7))
                    nc.vector.tensor_copy(
                        v_sb[:, tb, :, 0:64].bitcast(F32R),
                        pv.rearrange("p (i d) -> p i d", i=8))

                for tb_rel in range(4):
                    yield (lambda tb_rel=tb_rel, vg=v_group: vg(tb_rel))

            if qkv:
                # Prologue: A(0) fully.
                for u in a_units(0):
                    u()

            for tcx in range(NTC):
                if qkv:
                    qts = qts_all[tcx]
                else:
                    # timing-isolation: read weights as stand-in q/k data
                    qts = [wq_sb[:, j] for j in range(4)]
                nxt = (list(a_units(tcx + 1))
                       if (qkv and weave and tcx + 1 < NTC) else [])
                if nxt and not pace:
                    for u in nxt:
                        u()
                    nxt = []
                nkb = 4 * tcx + 4
                n_slots = 4 * nkb
                emitted = 0
                slot = 0

                yt = yT_pool.tile([128, 4, 512], F32R, tag="yT",
                                  name=f"yt{rep}_{tcx}")
                if not attn:
                    for u in nxt:
                        u()
                    if proj:
                        nc.vector.memset(yt, 0.001)
                    _run_proj(tcx, yt, rep)
                    continue
                for j in range(4):
                    Y = [ps_y.tile([66, 512], F32, tag="y",
                                   name=f"Y{rep}_{tcx}_{j}_{h}")
                         for h in range(2)]
                    for kb in range(nkb):
                        want = ((slot + 1) * len(nxt)) // n_slots
                        while emitted < want:
                            nxt[emitted]()
                            emitted += 1
                        slot += 1

                        jj = kb - 4 * tcx
                        diag = jj >= 0
                        c0 = max(jj, 0) * 128
                        S2 = ps_s.tile([128, 1024], F32, tag="s",
                                       name=f"S{rep}_{tcx}_{j}_{kb}")
                        kT_src = (kT_sb[:, j, kb * 128:(kb + 1) * 128]
                                  if qkv else
                                  wk_sb[:, j, (kb % 4) * 128:(kb % 4 + 1) * 128])
                        for h in range(2):
                            nc.tensor.matmul(
                                S2[:, h * 512 + c0:h * 512 + 512],
                                kT_src[h * 64:(h + 1) * 64],
                                qts[j][h * 64:(h + 1) * 64, c0:512],
                                start=True, stop=not diag,
                                tile_position=(h * 64, 0))
                        if diag:
                            # S[:, h*512+c0 : +128] += triT.T @ I (causal mask)
                            sv = S2.rearrange("p (h q) -> p h q", h=2)[
                                :, :, c0:c0 + 128]
                            nc.tensor.matmul(
                                sv, triT_sb,
                                id2_sb.rearrange("p (h q) -> p h q", h=2),
                                start=False, stop=True, skip_group_check=True)
                        att2 = attT_pool.tile([128, 1024], F32R, tag="attT",
                                              name=f"attT{rep}_{tcx}_{j}_{kb}")
                        nc.scalar.activation(
                            att2.rearrange("p (h q) -> p h q", h=2)[:, :, c0:512],
                            S2.rearrange("p (h q) -> p h q", h=2)[:, :, c0:512],
                            AF.Exp)
                        for h in range(2):
                            nc.tensor.matmul(
                                Y[h][:, c0:512],
                                v_sb[:, kb, 2 * j + h, :].bitcast(F32R),
                                att2[:, h * 512 + c0:h * 512 + 512],
                                start=(kb == 0), stop=(kb == nkb - 1))
                    for h in range(2):
                        rc = rc_pool.tile([1, 512], F32, tag="rc",
                                          name=f"rc{rep}_{tcx}_{j}_{h}")
                        nc.vector.reciprocal(rc, Y[h][64:65, :])
                        bcs = bcs_pool.tile([64, 512], F32, tag="bcs",
                                            name=f"bcs{rep}_{tcx}_{j}_{h}")
                        nc.gpsimd.partition_broadcast(bcs, rc)
                        nc.vector.tensor_mul(yt[h * 64:(h + 1) * 64, j, :],
                                             Y[h][0:64, :], bcs)
                while emitted < len(nxt):
                    nxt[emitted]()
                    emitted += 1
                _run_proj(tcx, yt, rep)

    nc.compile()
    return nc


def _get_nc():
    global _CACHED_NC
    if _CACHED_NC is None:
        _CACHED_NC = _build()
    return _CACHED_NC


def make_in_maps(x, W_attn, b_attn, W_proj):
    x = np.asarray(x, np.float32)
    W_attn = np.asarray(W_attn, np.float32)
    b_attn = np.asarray(b_attn, np.float32)
    scale = np.float32(1.0 / np.sqrt(D))
    # triT[q, k] = 0 if k <= q else -1e4  (mask added as triT.T @ I)
    triT = np.where(np.arange(128)[None, :] <= np.arange(128)[:, None],
                    np.float32(0.0), np.float32(-1e4)).astype(np.float32)
    id2 = np.concatenate([np.eye(128, dtype=np.float32)] * 2, axis=1)
    xts = [np.ascontiguousarray(x[b].T) for b in range(B)]
    in_maps = []
    for core in range(NCORES):
        b = core // 2
        hs = (core % 2) * FL
        qc = slice(hs, hs + FL)
        kc = slice(C + hs, C + hs + FL)
        vc = slice(2 * C + hs, 2 * C + hs + FL)
        in_maps.append({
            "xt": xts[b],
            "wq": np.ascontiguousarray(W_attn[:, qc] * scale),
            "wk": np.ascontiguousarray(W_attn[:, kc]),
            "wv": np.ascontiguousarray(W_attn[:, vc]),
            "wp": np.ascontiguousarray(np.asarray(W_proj, np.float32)[hs:hs + FL, :]),
            "bq": np.ascontiguousarray((b_attn[qc] * scale).reshape(4, 128).T),
            "bk": np.ascontiguousarray(b_attn[kc].reshape(4, 128).T),
            "triT": triT,
            "id2": id2,
        })
    return in_maps


def kernel(x, W_attn, b_attn, W_proj, b_proj):
    x = np.asarray(x, np.float32)
    W_attn = np.asarray(W_attn, np.float32)
    b_attn = np.asarray(b_attn, np.float32)
    W_proj = np.asarray(W_proj, np.float32)
    b_proj = np.asarray(b_proj, np.float32)

    nc = _get_nc()
    in_maps = make_in_maps(x, W_attn, b_attn, W_proj)
    res = run_bass_kernel_spmd(nc, in_maps, list(range(NCORES)))
    outs = [res.results[i]["out"] for i in range(NCORES)]
    y = np.stack([outs[2 * b] + outs[2 * b + 1] for b in range(B)])
    # v-bias folds through attention (rows sum to 1) into a constant output
    # bias: b_proj + b_v @ W_proj.
    bias_out = b_proj + b_attn[2 * C:] @ W_proj
    return (y + bias_out[None, None, :]).astype(np.float32)
